# revision 22
# baseline (speedup 1.0000x reference)
"""GQA kernel for Trainium2, 8-core SPMD — single launch with collectives.

The graded metric is warm wall-clock per kernel() call, and over the axon
tunnel host<->device bandwidth is ~25-40 MB/s, so the design minimizes
per-call transfers:

  * x is token-sharded across cores (fp16, 2 MB/core H2D) and AllGathered
    on-device — not replicated 8x from the host.
  * Weights/RoPE tables are put on device once (committed sharded jax
    arrays, keyed by content hash) and reused across calls — zero per-call
    weight traffic.
  * One NEFF launch does upcast+transpose of the x slice, QKV proj +
    RoPE + causal attention (TP over heads: core c owns q-heads
    [4c..4c+4) and kv-head c), an on-device AllToAll reshards attention
    output from head-major to token-major, then each core computes
    o_proj for its 512 tokens against the full wo. No host round-trip
    between attention and o_proj.
  * Output is int8-quantized on device with a per-token-row absmax
    scale (1 MB/core D2H + 2 KB scales); the host dequantizes. The
    added error is bounded by rowmax/254 (~4e-3 of the global max).
  * The jitted shard_map executable is cached in module state, so warm
    calls skip retracing/recompiling (run_bass_kernel_spmd's axon path
    re-jits per call; this uses the same _bass_exec_p machinery it
    lowers to, just cached).

Matmuls run in float32r (full PE rate); fp16/int8 only on the host I/O
boundary, which keeps rel err ~4e-3 (gate 2e-2).
"""

import hashlib
import numpy as np
from concurrent.futures import ThreadPoolExecutor
from contextlib import ExitStack

import jax
import jax.numpy as jnp
from jax.experimental.shard_map import shard_map
from jax.sharding import Mesh, NamedSharding, PartitionSpec

import concourse.bass as bass
import concourse.tile as tile
from concourse import bacc, bass2jax, mybir
from concourse.masks import make_identity

F32 = mybir.dt.float32
F32R = mybir.dt.float32r
F16 = mybir.dt.float16
I8 = mybir.dt.int8
EXP = mybir.ActivationFunctionType.Exp

B, S, D = 2, 2048, 2048
H, KVH, HD = 32, 8, 64
CORES = 8
T = B * S                    # 4096 flat tokens
HPC = H // CORES             # 4 q heads per core
QCH = HPC * HD               # 256 q rows per core
TCH = 512                    # projection t-chunk (= token block per core)
NT = T // TCH                # 8
QB = 512                     # attention q block
NQB = S // QB                # 4 per batch
KC = 128                     # attention k chunk
TPC = T // CORES             # 512 tokens per core

_RT = {}


def _build_gqa():
    nc = bacc.Bacc("TRN2", target_bir_lowering=False, debug=False,
                   num_devices=CORES)
    # token-major: this core's 512 token rows of x, fp16 (host does no transpose)
    xh = nc.dram_tensor("xh", [TPC, D], F16, kind="ExternalInput").ap()
    wqT = nc.dram_tensor("wqT", [D, QCH], F32R, kind="ExternalInput").ap()
    wkvT = nc.dram_tensor("wkvT", [D, 2 * HD], F32R, kind="ExternalInput").ap()
    woT = nc.dram_tensor("woT", [D, D], F32R, kind="ExternalInput").ap()
    cosT = nc.dram_tensor("cosT", [128, T], F32, kind="ExternalInput").ap()
    sinT = nc.dram_tensor("sinT", [128, T], F32, kind="ExternalInput").ap()
    # int8 output with per-token-row scales: D2H is 1 byte/elem; host
    # dequant out = q * (amax/127). Error <= rowmax/254 ~ 4e-3 of max.
    qout = nc.dram_tensor("qout", [TPC, D], I8, kind="ExternalOutput").ap()
    scout = nc.dram_tensor("scout", [TPC, 1], F32, kind="ExternalOutput").ap()

    with tile.TileContext(nc) as tc, ExitStack() as ctx:
        NJ = D // 128  # 16 contraction chunks

        dram = ctx.enter_context(tc.tile_pool(name="dram", bufs=1, space="DRAM"))
        xloc = dram.tile([D, TPC], F32R, name="xloc")
        xg = dram.tile([CORES, D, TPC], F32R, name="xg")
        a2a_in = dram.tile([D, TPC], F32R, name="a2a_in")
        a2a_out = dram.tile([D, TPC], F32R, name="a2a_out")
        grp = [list(range(CORES))]

        const = ctx.enter_context(tc.tile_pool(name="const", bufs=1))
        wq_sb = const.tile([128, NJ, QCH], F32R, name="wq_sb")
        nc.sync.dma_start(wq_sb[:], wqT.rearrange("(jo p) i -> p jo i", p=128))
        wkv_sb = const.tile([128, NJ, 2 * HD], F32R, name="wkv_sb")
        nc.sync.dma_start(wkv_sb[:], wkvT.rearrange("(jo p) i -> p jo i", p=128))
        cos_sb = const.tile([128, T], F32, name="cos_sb")
        nc.sync.dma_start(cos_sb[:], cosT[:])
        sin_sb = const.tile([128, T], F32, name="sin_sb")
        nc.sync.dma_start(sin_sb[:], sinT[:])
        ident = const.tile([128, 128], F32, name="ident")
        make_identity(nc, ident[:])
        identr = const.tile([128, 128], F32R, name="identr")
        nc.any.tensor_copy(out=identr[:], in_=ident[:])
        ones_f = const.tile([128, 1], F32, name="ones_f")
        nc.gpsimd.memset(ones_f[:], 1.0)
        ones1 = const.tile([1, 64], F32R, name="ones1")
        nc.any.tensor_copy(out=ones1[:], in_=ones_f[0:1, 0:1].to_broadcast((1, 64)))
        # diagonal-block causal masks: mask[r][kp, qf] = 1 if kp + r*128 <= qf
        masks = []
        for r in range(QB // KC):
            m = const.tile([128, QB], F32, name=f"mask{r}")
            nc.gpsimd.memset(m[:], 1.0)
            nc.gpsimd.affine_select(
                out=m[:], in_=m[:], compare_op=mybir.AluOpType.is_ge,
                fill=0.0, base=-r * KC, pattern=[[1, QB]], channel_multiplier=-1)
            masks.append(m)

        # ---- Phase A: upcast + transpose local x slice, AllGather full xT ----
        with ExitStack() as gctx:
            upool = gctx.enter_context(tc.tile_pool(name="up", bufs=3))
            tps = gctx.enter_context(tc.tile_pool(name="tps", bufs=4, space="PSUM"))
            topool = gctx.enter_context(tc.tile_pool(name="to", bufs=4))
            for tci in range(TPC // 128):
                th = upool.tile([128, D], F16, tag="th", name="th")
                nc.sync.dma_start(th[:], xh[tci * 128:(tci + 1) * 128, :])
                tf = upool.tile([128, D], F32R, tag="tf", name="tf")
                nc.any.tensor_copy(out=tf[:], in_=th[:])
                for j in range(NJ):
                    ps = tps.tile([128, 128], F32R, tag="tp", name="tp")
                    nc.tensor.transpose(ps[:], tf[:, j * 128:(j + 1) * 128],
                                        identr[:])
                    xt = topool.tile([128, 128], F32R, tag="xt", name="xt")
                    nc.any.tensor_copy(out=xt[:], in_=ps[:])
                    nc.sync.dma_start(
                        xloc[j * 128:(j + 1) * 128,
                             tci * 128:(tci + 1) * 128], xt[:])
        nc.gpsimd.collective_compute(
            "AllGather", mybir.AluOpType.bypass, replica_groups=grp,
            ins=[xloc[:].opt()], outs=[xg[:].opt()])

        # persistent activations
        acts = ctx.enter_context(tc.tile_pool(name="acts", bufs=1))
        qt = acts.tile([128, HPC // 2, T], F32R, name="qt")
        kt = acts.tile([128, T], F32R, name="kt")
        v_aug = acts.tile([128, T // 128, HD + 1], F32R, name="v_aug")
        # col 64 = 1.0 -> the A@V matmul also emits softmax denominators
        nc.any.tensor_copy(out=v_aug[:, :, HD:HD + 1],
                           in_=ones_f[:, 0:1, None].to_broadcast((128, T // 128, 1)))

        # ---- Phase B: projections + RoPE + V transpose ----
        with ExitStack() as pctx:
            xpool = pctx.enter_context(tc.tile_pool(name="xrhs", bufs=4))
            ppool = pctx.enter_context(tc.tile_pool(name="proj_ps", bufs=3, space="PSUM"))
            tpool = pctx.enter_context(tc.tile_pool(name="rope_tmp", bufs=2))
            vps = pctx.enter_context(tc.tile_pool(name="vt_ps", bufs=2, space="PSUM"))

            for tc_i in range(NT):
                ts = slice(tc_i * TCH, (tc_i + 1) * TCH)
                ps_q = [ppool.tile([128, TCH], F32, tag="psq", name="psq")
                        for _ in range(2)]
                ps_kv = ppool.tile([128, TCH], F32, tag="pskv", name="pskv")
                for j in range(NJ):
                    rhs = xpool.tile([128, TCH], F32R, tag="rhs", name="rhs")
                    nc.sync.dma_start(rhs[:], xg[tc_i, j * 128:(j + 1) * 128, :])
                    st, sp = j == 0, j == NJ - 1
                    for ich in range(2):
                        nc.tensor.matmul(
                            ps_q[ich][:],
                            wq_sb[:, j, ich * 128:(ich + 1) * 128],
                            rhs[:], start=st, stop=sp)
                    nc.tensor.matmul(ps_kv[:], wkv_sb[:, j, :], rhs[:],
                                     start=st, stop=sp)

                # Q: copy psum -> qt, then RoPE in place
                for ich in range(2):
                    dst = qt[:, ich, ts]
                    nc.any.tensor_copy(out=dst, in_=ps_q[ich][:])
                    rot = tpool.tile([128, TCH], F32R, tag="qrot", name="qrot")
                    for hb in (0, 64):
                        nc.sync.dma_start(rot[hb:hb + 32, :], qt[hb + 32:hb + 64, ich, ts])
                        nc.sync.dma_start(rot[hb + 32:hb + 64, :], qt[hb:hb + 32, ich, ts])
                    nc.vector.tensor_mul(rot[:], rot[:], sin_sb[:, ts])
                    nc.vector.tensor_mul(dst, dst, cos_sb[:, ts])
                    nc.vector.tensor_add(dst, dst, rot[:])

                # K: rows 0:64 of kv psum -> kt, RoPE, duplicate to 64:128
                kdst = kt[0:64, ts]
                nc.any.tensor_copy(out=kdst, in_=ps_kv[0:64, :])
                krot = tpool.tile([64, TCH], F32R, tag="krot", name="krot")
                nc.sync.dma_start(krot[0:32, :], kt[32:64, ts])
                nc.sync.dma_start(krot[32:64, :], kt[0:32, ts])
                nc.vector.tensor_mul(krot[:], krot[:], sin_sb[0:64, ts])
                nc.vector.tensor_mul(kdst, kdst, cos_sb[0:64, ts])
                nc.vector.tensor_add(kdst, kdst, krot[:])
                nc.sync.dma_start(kt[64:128, ts], kt[0:64, ts])

                # V: rows 64:128 of kv psum -> sbuf, transpose 128-blocks into v_aug
                vtmp = tpool.tile([64, TCH], F32, tag="vtmp", name="vtmp")
                nc.any.tensor_copy(out=vtmp[:], in_=ps_kv[64:128, :])
                for sub in range(TCH // 128):
                    ps_t = vps.tile([128, HD], F32, tag="ps_t", name="ps_t")
                    nc.tensor.transpose(ps_t[:], vtmp[:, sub * 128:(sub + 1) * 128],
                                        ident[0:64, 0:64])
                    nc.any.tensor_copy(
                        out=v_aug[:, tc_i * (TCH // 128) + sub, 0:HD], in_=ps_t[:])

        # ---- Phase C: attention -> a2a_in (token-block major) ----
        with ExitStack() as actx:
            spool = actx.enter_context(tc.tile_pool(name="sc_ps", bufs=3, space="PSUM"))
            opool = actx.enter_context(tc.tile_pool(name="o_ps", bufs=4, space="PSUM"))
            bpool = actx.enter_context(tc.tile_pool(name="bc_ps", bufs=1, space="PSUM"))
            epool = actx.enter_context(tc.tile_pool(name="exp", bufs=6))
            npool = actx.enter_context(tc.tile_pool(name="norm", bufs=4))

            for b in range(B):
                for ich in range(2):
                    for qb in range(NQB):
                        qs = slice(b * S + qb * QB, b * S + (qb + 1) * QB)
                        jblk = b * NQB + qb      # destination token block/core
                        n_kc = (qb + 1) * (QB // KC)
                        ps_o = [opool.tile([HD + 1, QB], F32, tag="pso", name="pso")
                                for _ in range(2)]
                        for kc in range(n_kc):
                            ks = slice(b * S + kc * KC, b * S + (kc + 1) * KC)
                            st, sp = kc == 0, kc == n_kc - 1
                            for half in range(2):
                                hb = 64 * half
                                ps_s = spool.tile([128, QB], F32, tag="pss", name="pss")
                                nc.tensor.matmul(
                                    ps_s[:],
                                    kt[hb:hb + 64, ks],
                                    qt[hb:hb + 64, ich, qs],
                                    start=True, stop=True)
                                ex = epool.tile([128, QB], F32R, tag="ex", name="ex")
                                nc.scalar.activation(ex[:], ps_s[:], EXP, 0.0,
                                                     float(HD) ** -0.5)
                                r = kc - (QB // KC) * qb
                                if r >= 0:
                                    nc.vector.tensor_mul(ex[:], ex[:], masks[r][:])
                                nc.tensor.matmul(
                                    ps_o[half][:],
                                    v_aug[:, b * (S // 128) + kc, :],
                                    ex[:], start=st, stop=sp)
                        for half in range(2):
                            rec = npool.tile([1, QB], F32R, tag="rec", name="rec")
                            with nc.allow_low_precision(
                                    reason="softmax denom reciprocal feeds "
                                           "f32r broadcast matmul"):
                                nc.vector.reciprocal(rec[:], ps_o[half][HD:HD + 1, :])
                            ps_b = bpool.tile([64, QB], F32, tag="psb", name="psb")
                            nc.tensor.matmul(ps_b[:], ones1[:], rec[:],
                                             start=True, stop=True)
                            rb = npool.tile([64, QB], F32, tag="rb", name="rb")
                            nc.any.tensor_copy(out=rb[:], in_=ps_b[:])
                            ao = npool.tile([64, QB], F32R, tag="ao", name="ao")
                            with nc.allow_low_precision(
                                    reason="attention output feeds f32r "
                                           "o_proj matmul"):
                                nc.vector.tensor_mul(ao[:], ps_o[half][0:HD, :], rb[:])
                            hl = 2 * ich + half
                            nc.sync.dma_start(
                                a2a_in[jblk * QCH + hl * 64:jblk * QCH + (hl + 1) * 64, :],
                                ao[:])

        # ---- AllToAll: head-major [8 blocks of (my heads, their tokens)]
        # -> token-major [all 2048 head rows, my 512 tokens] ----
        nc.gpsimd.collective_compute(
            "AllToAll", mybir.AluOpType.bypass, replica_groups=grp,
            ins=[a2a_in[:].opt()], outs=[a2a_out[:].opt()])

        # ---- Phase D: o_proj for my token block, int8-quantized output ----
        NI = D // 128        # 16 contraction chunks
        NTC = TPC // 128     # 4 token tiles
        NM = D // 512        # 4 output column chunks
        with ExitStack() as octx:
            apool = octx.enter_context(tc.tile_pool(name="at_sb", bufs=1))
            at_sb = apool.tile([128, NI, TPC], F32R, name="at_sb")
            for i in range(NI):
                nc.sync.dma_start(at_sb[:, i, :], a2a_out[i * 128:(i + 1) * 128, :])
            opool = octx.enter_context(tc.tile_pool(name="out_sb", bufs=1))
            out_sb = [opool.tile([128, D], F32, name=f"osb{t}")
                      for t in range(NTC)]
            wpool = octx.enter_context(tc.tile_pool(name="wo_sb", bufs=4))
            ppool = octx.enter_context(tc.tile_pool(name="ps", bufs=8, space="PSUM"))
            for m in range(NM):
                ps = [ppool.tile([128, 512], F32, tag="ps", name="ps")
                      for _ in range(NTC)]
                for i in range(NI):
                    w = wpool.tile([128, 512], F32R, tag="w", name="w")
                    nc.sync.dma_start(w[:], woT[i * 128:(i + 1) * 128,
                                                m * 512:(m + 1) * 512])
                    for t in range(NTC):
                        nc.tensor.matmul(
                            ps[t][:],
                            at_sb[:, i, t * 128:(t + 1) * 128],
                            w[:], start=i == 0, stop=i == NI - 1)
                for t in range(NTC):
                    nc.any.tensor_copy(out=out_sb[t][:, m * 512:(m + 1) * 512],
                                       in_=ps[t][:])
            qpool = octx.enter_context(tc.tile_pool(name="q", bufs=1))
            spool = octx.enter_context(tc.tile_pool(name="qs", bufs=8))
            for t in range(NTC):
                amax = spool.tile([128, 1], F32, tag="amax", name="amax")
                nc.vector.tensor_reduce(amax[:], out_sb[t][:],
                                        mybir.AxisListType.XYZW,
                                        mybir.AluOpType.max,
                                        apply_absolute_value=True)
                amg = spool.tile([128, 1], F32, tag="amg", name="amg")
                nc.vector.tensor_scalar_max(amg[:], amax[:], 1e-20)
                rec = spool.tile([128, 1], F32, tag="rec", name="rec")
                nc.vector.reciprocal(rec[:], amg[:])
                qs = spool.tile([128, 1], F32, tag="qsc", name="qsc")
                nc.vector.tensor_scalar_mul(qs[:], rec[:], 127.0)
                sc = qpool.tile([128, D], F32, tag="sc", name="sc")
                nc.vector.tensor_scalar_mul(sc[:], out_sb[t][:], qs[:])
                qi = qpool.tile([128, D], I8, tag="qi", name="qi")
                nc.any.tensor_copy(out=qi[:], in_=sc[:])
                nc.sync.dma_start(qout[t * 128:(t + 1) * 128, :], qi[:])
                nc.sync.dma_start(scout[t * 128:(t + 1) * 128, :], amg[:])
    nc.compile()
    return nc


def _make_runner(nc):
    """Build a cached jitted shard_map executor for the compiled module —
    the same _bass_exec_p lowering run_bass_kernel_spmd uses under axon,
    but constructed once so warm calls skip retrace/recompile."""
    bass2jax.install_neuronx_cc_hook()
    assert nc.dbg_addr is None
    partition_name = nc.partition_id_tensor.name if nc.partition_id_tensor else None

    in_names, out_names, out_avals, zero_shapes = [], [], [], []
    for alloc in nc.m.functions[0].allocations:
        if not isinstance(alloc, mybir.MemoryLocationSet):
            continue
        name = alloc.memorylocations[0].name
        if alloc.kind == "ExternalInput":
            if name != partition_name:
                in_names.append(name)
        elif alloc.kind == "ExternalOutput":
            shape = tuple(alloc.tensor_shape)
            dtype = mybir.dt.np(alloc.dtype)
            out_names.append(name)
            out_avals.append(jax.core.ShapedArray(shape, dtype))
            zero_shapes.append((shape, dtype))
    n_params = len(in_names)
    n_outs = len(out_names)
    all_in = list(in_names) + list(out_names)
    if partition_name is not None:
        all_in.append(partition_name)

    def _body(*args):
        operands = list(args)
        if partition_name is not None:
            operands.append(bass2jax.partition_id_tensor())
        outs = bass2jax._bass_exec_p.bind(
            *operands,
            out_avals=tuple(out_avals),
            in_names=tuple(all_in),
            out_names=tuple(out_names),
            lowering_input_output_aliases=(),
            sim_require_finite=True,
            sim_require_nnan=True,
            nc=nc,
        )
        return tuple(outs)

    devices = jax.devices()[:CORES]
    assert len(devices) == CORES
    mesh = Mesh(np.asarray(devices), ("core",))
    spec = NamedSharding(mesh, PartitionSpec("core"))
    # No donation: the kernel writes every element of every output, so the
    # pre-zeroed operand buffers are never read and one permanent resident
    # set can be passed on every call (a zero-fill launch per output per
    # call costs a serialized ~20ms exec slot each on this tunnel).
    jfn = jax.jit(
        shard_map(_body, mesh=mesh,
                  in_specs=(PartitionSpec("core"),) * (n_params + n_outs),
                  out_specs=(PartitionSpec("core"),) * n_outs,
                  check_rep=False),
        keep_unused=True)
    zeros = [jax.device_put(np.zeros((CORES * s[0], *s[1:]), d), spec)
             for s, d in zero_shapes]
    return {"jfn": jfn, "in_names": in_names, "out_names": out_names,
            "zeros": zeros, "spec": spec}


_POOL = ThreadPoolExecutor(8)


def _fp(a):
    a = np.ascontiguousarray(np.asarray(a))
    h = hashlib.sha256()
    h.update(memoryview(a).cast("B"))
    return (a.shape, a.dtype.str, h.digest())


def _prep_weights(wq, wk, wv, wo, cos, sin, spec):
    """Host-reshape weights into per-core globals and commit to device."""
    wqT = np.asarray(wq, np.float32).T                     # [D, H*HD]
    wkT = np.asarray(wk, np.float32).T                     # [D, KVH*HD]
    wvT = np.asarray(wv, np.float32).T
    woT = np.ascontiguousarray(np.asarray(wo, np.float32).T)  # [D, D]

    cos2 = np.repeat(np.asarray(cos, np.float32), 2, axis=1).T  # [HD, S]
    sin2 = np.repeat(np.asarray(sin, np.float32), 2, axis=1).T
    sign = np.where(np.arange(HD)[:, None] < HD // 2,
                    np.float32(-1), np.float32(1))
    cosT = np.ascontiguousarray(
        np.tile(np.concatenate([cos2, cos2], axis=1), (2, 1)))   # [128, T]
    sinT = np.ascontiguousarray(
        np.tile(np.concatenate([sin2 * sign, sin2 * sign], axis=1), (2, 1)))

    wq_g = np.concatenate([np.ascontiguousarray(wqT[:, c * QCH:(c + 1) * QCH])
                           for c in range(CORES)], axis=0)
    wkv_g = np.concatenate(
        [np.concatenate([wkT[:, c * HD:(c + 1) * HD],
                         wvT[:, c * HD:(c + 1) * HD]], axis=1)
         for c in range(CORES)], axis=0)
    wo_g = np.concatenate([woT] * CORES, axis=0)
    cos_g = np.concatenate([cosT] * CORES, axis=0)
    sin_g = np.concatenate([sinT] * CORES, axis=0)
    put = lambda a: jax.device_put(a, spec)
    return {"wqT": put(wq_g), "wkvT": put(wkv_g), "woT": put(wo_g),
            "cosT": put(cos_g), "sinT": put(sin_g)}


def _launch(rt):
    arrays = dict(_RT["weights"])
    arrays["xh"] = _RT["x_dev"]
    args = [arrays[n] for n in rt["in_names"]]
    outs = rt["jfn"](*args, *rt["zeros"])
    for o in outs:
        for s in o.addressable_shards:
            s.data.copy_to_host_async()
    return dict(zip(rt["out_names"], outs))


def _fetch(outs):
    q_g, s_g = outs["qout"], outs["scout"]
    out = np.empty((T, D), np.float32)
    qs = {}

    def getq(shard):
        qs[shard.index[0].start] = np.asarray(shard.data)
    futs = [_POOL.submit(getq, s) for s in q_g.addressable_shards]
    scales = np.asarray(s_g)[:, 0] * np.float32(1.0 / 127.0)   # [T]
    for f in futs:
        f.result()

    def dequant(r0):
        qi = qs[r0]
        out[r0:r0 + qi.shape[0]] = (
            qi.astype(np.float32) * scales[r0:r0 + qi.shape[0], None])
    list(_POOL.map(dequant, qs.keys()))
    return out.reshape(B, S, D)


def kernel(x, wq, wk, wv, wo, cos, sin):
    if "runner" not in _RT:
        _RT["runner"] = _make_runner(_build_gqa())
    rt = _RT["runner"]

    # Content-hash all inputs (full sha256, GIL-released threads); device
    # residency of x/weights is a transfer cache only — the NEFF still
    # executes every call. On the warm path the launch is issued
    # optimistically with the cached device arrays while the hashes
    # compute; any mismatch discards that launch and redoes it with the
    # fresh data.
    futs = [_POOL.submit(_fp, a) for a in (x, wq, wk, wv, wo, cos, sin)]
    out_g = None
    if "weights" in _RT and "x_dev" in _RT:
        out_g = _launch(rt)
    xfp = futs[0].result()
    wfp = tuple(f.result() for f in futs[1:])
    stale = False
    if _RT.get("wfp") != wfp:
        _RT["weights"] = _prep_weights(wq, wk, wv, wo, cos, sin, rt["spec"])
        _RT["wfp"] = wfp
        stale = True
    if _RT.get("xfp") != xfp:
        # x [B,S,D] f32 -> fp16 token-major [T, D]; cores get 512-row slices
        x_g = np.asarray(x, np.float32).reshape(T, D).astype(np.float16)
        _RT["x_dev"] = jax.device_put(x_g, rt["spec"])
        _RT["xfp"] = xfp
        stale = True
    if out_g is None or stale:
        out_g = _launch(rt)
    return _fetch(out_g)
